# revision 12
# baseline (speedup 1.0000x reference)
"""Trainium2 Bass kernel for nn_ConservativeDynamicCurvatureMLP.

Data-parallel over 8 NeuronCores: batch (8192) sharded into 8 shards of
1024 rows; weights replicated.  The curvature scalar couples shards via a
single-scalar AllReduce.

Math (reference):
    h = tanh(x @ W1 + b1)
    u = sigmoid(h @ W2 + b2)
    c = clip(mean(MIN_C + (MAX_C-MIN_C) * sigmoid(relu(x@cp_w1.T+cp_b1)@cp_w2.T+cp_b2)), MIN_C, MAX_C)
    z = poincare_ball_layer(h, u, c, T)   ==  alpha(row)*h + beta(row)*u
    out = z @ Wo + bo

Performance structure (v2):
  * MM1 (x@W1) in bf16, feature-major, as before.
  * MM2 (h@W2) in fp8 e4m3 with DoubleRow perf mode: 2 k-slices per
    matmul -> 2x PE throughput.  h is cast bf16->e4m3 on DVE (direct cast,
    |h|<=1), W2 is host-prescaled by 256 into e4m3 (max |W2*256| ~ 28 << 240);
    the 1/256 is folded into the sigmoid activation scale.  Predicted
    end-to-end rel err ~1.3e-2 (gate 2e-2), simulated with exact RNE casts.
  * MM2 runs column-chunk-major (two 512-column halves): the per-half
    stats -> scalar-chain -> z-combine tail overlaps the other half's
    matmuls / the output projection, keeping the PE dense (no HAM
    re-throttle).
  * MMo (z@Wo) in bf16, och-wave structure, overlapped with the half-1
    z-combine.
"""

import tempfile
from contextlib import ExitStack

import numpy as np
import ml_dtypes

import concourse.bass as bass
import concourse.bacc as bacc
import concourse.mybir as mybir
import concourse.tile as tile
from concourse.bass_utils import run_bass_kernel_spmd

P = 128
N_CORES = 8
B_FULL = 8192
BL = B_FULL // N_CORES          # 1024 rows per core
IN = 3072
HID = 4096
OUT = 1000
KI = IN // P                    # 24
KH = HID // P                   # 32
KP = KH // 2                    # 16 DoubleRow k-pairs
MIN_C = 0.001 * 0.5
MAX_C = 0.001 * 2.0
T_CONST = 0.7
EPS = 1e-7
W2_SCALE = 256.0

dt = mybir.dt
AF = mybir.ActivationFunctionType
ALU = mybir.AluOpType
DR = mybir.MatmulPerfMode.DoubleRow
BF = ml_dtypes.bfloat16
E4 = ml_dtypes.float8_e4m3fn

_nc_cache = []


def _build(with_b1, with_b2):
    nc = bacc.Bacc("TRN2", target_bir_lowering=False, debug=False,
                   num_devices=N_CORES)

    xT_d = nc.dram_tensor("xT", [KI, P, BL], dt.bfloat16, kind="ExternalInput")
    # w1r[mh, p, ki, q] = W1[ki*128+p, mh*128+q]
    w1_d = nc.dram_tensor("w1", [KH, P, KI, P], dt.bfloat16, kind="ExternalInput")
    # w2r[mh, p, kp, j, q] = W2[(2*kp+j)*128+p, mh*128+q] * 256  (fp8 pairs)
    w2_d = nc.dram_tensor("w2", [KH, P, KP, 2, P], dt.float8e4,
                          kind="ExternalInput")
    wo_d = nc.dram_tensor("wo", [KH, P, OUT], dt.bfloat16, kind="ExternalInput")
    cpw1_d = nc.dram_tensor("cpw1", [KI, P, 16], dt.bfloat16, kind="ExternalInput")
    cpw2_d = nc.dram_tensor("cpw2", [16, 1], dt.bfloat16, kind="ExternalInput")
    cpb1_d = nc.dram_tensor("cpb1", [16, 1], dt.float32, kind="ExternalInput")
    cpb2_d = nc.dram_tensor("cpb2", [1, 1], dt.float32, kind="ExternalInput")
    b1_d = nc.dram_tensor("b1", [P, KH], dt.float32, kind="ExternalInput") if with_b1 else None
    b2_d = nc.dram_tensor("b2", [P, KH], dt.float32, kind="ExternalInput") if with_b2 else None
    out_d = nc.dram_tensor("out", [BL, OUT], dt.float32, kind="ExternalOutput")

    f32 = dt.float32
    bf16 = dt.bfloat16
    fp8 = dt.float8e4

    with tile.TileContext(nc) as tc, ExitStack() as ctx:
        const = ctx.enter_context(tc.tile_pool(name="const", bufs=1))
        big = ctx.enter_context(tc.tile_pool(name="big", bufs=1))
        htp = ctx.enter_context(tc.tile_pool(name="htp", bufs=1))
        hqp = ctx.enter_context(tc.tile_pool(name="hqp", bufs=1))
        wp = ctx.enter_context(tc.tile_pool(name="wp", bufs=2))
        scr = ctx.enter_context(tc.tile_pool(name="scr", bufs=3))
        zscr = ctx.enter_context(tc.tile_pool(name="zscr", bufs=2))
        sacc = ctx.enter_context(tc.tile_pool(name="sacc", bufs=2))
        abp = ctx.enter_context(tc.tile_pool(name="abp", bufs=1))
        scal = ctx.enter_context(tc.tile_pool(name="scal", bufs=1))
        outp = ctx.enter_context(tc.tile_pool(name="outp", bufs=2))
        cpp = ctx.enter_context(tc.tile_pool(name="cpp", bufs=1))
        dram = ctx.enter_context(tc.tile_pool(name="dram", bufs=1, space="DRAM"))

        V = nc.vector
        S = nc.scalar

        def sc(name, shape=(P, 8), dtype=f32):
            return scal.tile(list(shape), dtype, name=name, tag=name)

        # ---------- persistent activations (feature-major) ----------
        ones = const.tile([P, 1], f32, name="ones")
        nc.vector.memset(ones, 1.0)
        xT_sb = big.tile([P, KI, BL], bf16, name="xT_sb", tag="big",
                         padded_shape=[P, KH, BL])
        w1row0 = wp.tile([P, KI, P], bf16, name="w1row", tag="w")
        nc.sync.dma_start(out=w1row0, in_=w1_d[0])
        for a, b in ((0, 2), (2, 4), (4, 8), (8, 12), (12, 16), (16, 20),
                     (20, 24)):
            nc.gpsimd.dma_start(
                out=xT_sb[:, a:b, :],
                in_=xT_d[a:b].rearrange("k p b -> p k b"))
        hT_sb = htp.tile([P, KH, BL], bf16, name="hT_sb")
        hq_sb = hqp.tile([P, KP, 2, BL], fp8, name="hq_sb")
        if with_b1:
            b1_sb = const.tile([P, KH], f32, name="b1_sb")
            nc.sync.dma_start(out=b1_sb, in_=b1_d[:, :])
        if with_b2:
            b2_sb = const.tile([P, KH], f32, name="b2_sb")
            nc.sync.dma_start(out=b2_sb, in_=b2_d[:, :])

        st_d = dram.tile([3, BL], f32, name="st_d")
        ab_d = dram.tile([2, BL], bf16, name="ab_d")
        alpha_b = abp.tile([P, BL], bf16, name="alpha_b")
        beta_b = abp.tile([P, BL], bf16, name="beta_b")

        with ExitStack() as ph1:
            mm = ph1.enter_context(tc.tile_pool(name="mm", bufs=2, space="PSUM"))
            stp = ph1.enter_context(tc.tile_pool(name="stp", bufs=2, space="PSUM"))
            # per-half stat psums; rows: x2 @ 0, y2 @ 32, xy @ 64
            stat_ps = [stp.tile([P, 512], f32, name=f"stat_ps{ch}",
                                tag="stat") for ch in range(2)]

            # ---------- MM1: hT = tanh(W1.T @ xT), hq cast, x2 stats ------
            x2a = sacc.tile([P, BL], f32, name="x2a", tag="sacc")
            with nc.named_scope("mm1"):
                for mh in range(KH):
                    ps = mm.tile([P, BL], f32, name="ps", tag="mm")
                    if mh == 0:
                        w1row = w1row0
                    else:
                        w1row = wp.tile([P, KI, P], bf16, name="w1row",
                                        tag="w")
                        nc.sync.dma_start(out=w1row, in_=w1_d[mh])
                    for ki in range(KI):
                        nc.tensor.matmul(ps[:, 0:512], lhsT=w1row[:, ki, :],
                                         rhs=xT_sb[:, ki, 0:512],
                                         start=(ki == 0), stop=(ki == KI - 1))
                        nc.tensor.matmul(ps[:, 512:BL], lhsT=w1row[:, ki, :],
                                         rhs=xT_sb[:, ki, 512:BL],
                                         start=(ki == 0), stop=(ki == KI - 1))
                    if with_b1:
                        S.activation(hT_sb[:, mh, :], ps, AF.Tanh,
                                     bias=b1_sb[:, mh:mh + 1])
                    else:
                        S.activation(hT_sb[:, mh, :], ps, AF.Tanh)
                    V.tensor_copy(hq_sb[:, mh // 2, mh % 2, :],
                                  hT_sb[:, mh, :])
                    hh = scr.tile([P, BL], bf16, name="hh", tag="hh")
                    S.activation(hh, hT_sb[:, mh, :], AF.Square)
                    if mh == 0:
                        V.tensor_copy(x2a, hh)
                    else:
                        V.tensor_add(x2a, x2a, hh)
                for ch in range(2):
                    sl = slice(ch * 512, (ch + 1) * 512)
                    nc.tensor.matmul(stat_ps[ch][0:1, :], lhsT=ones,
                                     rhs=x2a[:, sl], start=True, stop=True,
                                     skip_group_check=True)

            # ---------- curvature predictor + AllReduce ----------
            with nc.named_scope("cp"):
                cpw1_sb = const.tile([P, KI, 16], bf16, name="cpw1_sb")
                nc.sync.dma_start(out=cpw1_sb,
                                  in_=cpw1_d.rearrange("k p q -> p k q"))
                cpw2_sb = const.tile([16, 1], bf16, name="cpw2_sb")
                nc.sync.dma_start(out=cpw2_sb, in_=cpw2_d[:, :])
                cpb1_sb = const.tile([16, 1], f32, name="cpb1_sb")
                nc.sync.dma_start(out=cpb1_sb, in_=cpb1_d[:, :])
                cpb2_sb = const.tile([1, 1], f32, name="cpb2_sb")
                nc.sync.dma_start(out=cpb2_sb, in_=cpb2_d[:, :])
                cph_sb = cpp.tile([16, BL], fp8, name="cph_sb")
                for ch in range(2):
                    cps = mm.tile([16, 512], f32, name="cps", tag="mm")
                    for ki in range(KI):
                        nc.tensor.matmul(
                            cps, lhsT=cpw1_sb[:, ki, :],
                            rhs=xT_sb[:, ki, ch * 512:(ch + 1) * 512],
                            start=(ki == 0), stop=(ki == KI - 1))
                    S.activation(cph_sb[:, ch * 512:(ch + 1) * 512], cps,
                                 AF.Relu, bias=cpb1_sb)
                sparts = []
                for ch in range(2):
                    c2p = mm.tile([1, 512], f32, name="c2p", tag="mm")
                    nc.tensor.matmul(c2p, lhsT=cpw2_sb,
                                     rhs=cph_sb[:16, ch * 512:(ch + 1) * 512],
                                     start=True, stop=True)
                    cpw = cpp.tile([1, 512], bf16, name="cpw", tag="cpw")
                    spart = cpp.tile([1, 1], f32, name=f"spart{ch}",
                                     tag=f"spart{ch}")
                    S.activation(cpw, c2p, AF.Sigmoid, bias=cpb2_sb,
                                 accum_out=spart)
                    sparts.append(spart)
                s_loc = cpp.tile([1, 1], f32, name="s_loc")
                V.tensor_add(s_loc, sparts[0], sparts[1])
                cin = dram.tile([1, 1], f32, name="cin")
                cout = dram.tile([1, 1], f32, name="cout")
                nc.sync.dma_start(out=cin, in_=s_loc)
                nc.gpsimd.collective_compute(
                    "AllReduce", ALU.add,
                    replica_groups=[list(range(N_CORES))],
                    ins=[cin.opt()], outs=[cout.opt()])
                s_b = sc("s_b", (P, 1))
                nc.gpsimd.dma_start(out=s_b, in_=cout.to_broadcast([P, 1]))
                c_b = sc("c_b", (P, 1))
                V.tensor_scalar(out=c_b, in0=s_b,
                                scalar1=(MAX_C - MIN_C) / B_FULL,
                                scalar2=MIN_C, op0=ALU.mult, op1=ALU.add)
                V.tensor_scalar_min(out=c_b, in0=c_b, scalar1=MAX_C)
                V.tensor_scalar_max(out=c_b, in0=c_b, scalar1=MIN_C)
                negc_b = sc("negc_b", (P, 1))
                V.tensor_scalar_mul(out=negc_b, in0=c_b, scalar1=-1.0)
                twoc_b = sc("twoc_b", (P, 1))
                V.tensor_scalar_mul(out=twoc_b, in0=c_b, scalar1=2.0)
                neg2c_b = sc("neg2c_b", (P, 1))
                V.tensor_scalar_mul(out=neg2c_b, in0=c_b, scalar1=-2.0)
                c2_b = sc("c2_b", (P, 1))
                V.tensor_mul(c2_b, c_b, c_b)

        # still inside ph1 scope vars; reopened below for MM2
            # ---------- per-row scalar chain (batch-major [128, 4]) -------
            y2a = sacc.tile([P, BL], f32, name="y2a", tag="sacc")
            xya = sacc.tile([P, BL], f32, name="xya", tag="sacc")
            uT_sb = big.tile([P, KH, BL], bf16, name="uT_sb", tag="big")

            def scalar_chain(ch):
                hsl = slice(ch * 512, (ch + 1) * 512)

                def sch(name):
                    return sc(f"{name}_{ch}", (P, 4))

                x2 = sch("x2")
                y2 = sch("y2")
                xy = sch("xy")
                for i, t in enumerate((x2, y2, xy)):
                    nc.scalar.dma_start(
                        out=t, in_=st_d[i, hsl].rearrange("(j p) -> p j", p=P))
                w = sch("w")
                V.scalar_tensor_tensor(out=w, in0=xy, scalar=-2.0, in1=y2,
                                       op0=ALU.mult, op1=ALU.add)
                A1 = sch("A1")
                V.tensor_scalar(out=A1, in0=w, scalar1=c_b, scalar2=1.0,
                                op0=ALU.mult, op1=ALU.add)
                A2 = sch("A2")
                V.tensor_scalar(out=A2, in0=x2, scalar1=negc_b, scalar2=1.0,
                                op0=ALU.mult, op1=ALU.add)
                p1 = sch("p1")
                V.tensor_mul(p1, x2, y2)
                den = sch("den")
                V.tensor_scalar(out=den, in0=p1, scalar1=c2_b, scalar2=1.0,
                                op0=ALU.mult, op1=ALU.add)
                V.scalar_tensor_tensor(out=den, in0=xy, scalar=neg2c_b, in1=den,
                                       op0=ALU.mult, op1=ALU.add)
                V.tensor_scalar_add(out=den, in0=den, scalar1=EPS)
                D = sch("D")
                V.reciprocal(D, den)
                t1 = sch("t1")
                V.tensor_mul(t1, A1, A1)
                V.tensor_mul(t1, t1, x2)
                t2 = sch("t2")
                V.tensor_mul(t2, A1, A2)
                V.tensor_mul(t2, t2, xy)
                t3 = sch("t3")
                V.tensor_mul(t3, A2, A2)
                V.tensor_mul(t3, t3, y2)
                na2 = sch("na2")
                V.scalar_tensor_tensor(out=na2, in0=t2, scalar=-2.0, in1=t1,
                                       op0=ALU.mult, op1=ALU.add)
                V.tensor_add(na2, na2, t3)
                dsq = sch("dsq")
                V.tensor_mul(dsq, D, D)
                V.tensor_mul(na2, na2, dsq)
                # tm = tanh(T*artanh(sqrt(s)))/sqrt(s) with s = c*na2 is
                # analytic in s; deg-12 poly on s in [0.5, 0.996] (max rel
                # err 5.7e-4 over the c-envelope, 4.5e-5 in the actual
                # band) -- keeps the whole chain on DVE (no ACT table
                # thrash against MM2's sigmoids).
                s_t = sch("s_t")
                V.tensor_scalar(out=s_t, in0=na2, scalar1=c_b, scalar2=None,
                                op0=ALU.mult)
                V.tensor_scalar_min(out=s_t, in0=s_t, scalar1=0.996)
                V.tensor_scalar_max(out=s_t, in0=s_t, scalar1=0.5)
                V.tensor_scalar_add(out=s_t, in0=s_t, scalar1=-0.75)
                TM_POLY = (2227824.6408410813, 448871.7528227819,
                           -312401.2221121575, -56799.3889483669,
                           17050.8363088851, 2766.4915063889557,
                           -404.5213056958804, -51.417097735340924,
                           6.522608450512562, 1.2576027346248937,
                           0.3868927385367392, 0.32011781072746887,
                           0.839226248286217)
                tm = sch("tm")
                tmw = sch("tmw")
                V.tensor_scalar(out=tm, in0=s_t, scalar1=TM_POLY[0],
                                scalar2=TM_POLY[1], op0=ALU.mult, op1=ALU.add)
                for cf_ in TM_POLY[2:]:
                    V.tensor_mul(tmw, tm, s_t)
                    V.tensor_scalar_add(out=tm, in0=tmw, scalar1=cf_)
                s1_ = sch("s1_")
                V.tensor_mul(s1_, A1, x2)
                s2_ = sch("s2_")
                V.tensor_mul(s2_, A2, xy)
                ha = sch("ha")
                V.tensor_sub(ha, s2_, s1_)
                V.tensor_mul(ha, ha, D)
                hm = sch("hm")
                V.tensor_mul(hm, tm, ha)
                tsq = sch("tsq")
                V.tensor_mul(tsq, tm, tm)
                m2 = sch("m2")
                V.tensor_mul(m2, tsq, na2)
                w2s = sch("w2s")
                V.scalar_tensor_tensor(out=w2s, in0=hm, scalar=2.0, in1=m2,
                                       op0=ALU.mult, op1=ALU.add)
                B1 = sch("B1")
                V.tensor_scalar(out=B1, in0=w2s, scalar1=c_b, scalar2=1.0,
                                op0=ALU.mult, op1=ALU.add)
                p2 = sch("p2")
                V.tensor_mul(p2, x2, m2)
                den2 = sch("den2")
                V.tensor_scalar(out=den2, in0=p2, scalar1=c2_b, scalar2=1.0,
                                op0=ALU.mult, op1=ALU.add)
                V.scalar_tensor_tensor(out=den2, in0=hm, scalar=twoc_b, in1=den2,
                                       op0=ALU.mult, op1=ALU.add)
                V.tensor_scalar_add(out=den2, in0=den2, scalar1=EPS)
                D2 = sch("D2")
                V.reciprocal(D2, den2)
                g = sch("g")
                V.tensor_mul(g, A2, tm)
                V.tensor_mul(g, g, D)
                w3 = sch("w3")
                V.tensor_mul(w3, g, A1)
                V.tensor_sub(w3, B1, w3)
                alpha_bm = sc(f"alpha_bm_{ch}", (P, 4), bf16)
                V.tensor_mul(alpha_bm, w3, D2)
                w4 = sch("w4")
                V.tensor_mul(w4, g, A2)
                beta_bm = sc(f"beta_bm_{ch}", (P, 4), bf16)
                V.tensor_mul(beta_bm, w4, D2)
                # store + broadcast on the vector queue: zero cross-engine
                # wake latency into the zcomb consumers, and keeps the sync
                # queue free for weight streaming
                nc.scalar.dma_start(
                    out=ab_d[0, hsl].rearrange("(j p) -> p j", p=P),
                    in_=alpha_bm)
                nc.scalar.dma_start(
                    out=ab_d[1, hsl].rearrange("(j p) -> p j", p=P),
                    in_=beta_bm)
                nc.scalar.dma_start(out=alpha_b[:, hsl],
                                    in_=ab_d[0:1, hsl].to_broadcast([P, 512]))
                nc.scalar.dma_start(out=beta_b[:, hsl],
                                    in_=ab_d[1:2, hsl].to_broadcast([P, 512]))

            def mm2_mh(ch, mh):
                csl = slice(ch * 512, (ch + 1) * 512)
                ps = mm.tile([P, 512], f32, name="ps2", tag="mm")
                w2row = wp.tile([P, KP, 2, P], fp8, name="w2row", tag="w")
                nc.sync.dma_start(out=w2row, in_=w2_d[mh])
                for kp in range(KP):
                    nc.tensor.matmul(ps, lhsT=w2row[:, kp],
                                     rhs=hq_sb[:, kp, :, csl],
                                     start=(kp == 0), stop=(kp == KP - 1),
                                     perf_mode=DR)
                if with_b2:
                    S.activation(uT_sb[:, mh, csl], ps, AF.Sigmoid,
                                 bias=b2_sb[:, mh:mh + 1],
                                 scale=1.0 / W2_SCALE)
                else:
                    S.activation(uT_sb[:, mh, csl], ps, AF.Sigmoid,
                                 scale=1.0 / W2_SCALE)
                uu = scr.tile([P, 512], bf16, name="uu", tag="hh")
                S.activation(uu, uT_sb[:, mh, csl], AF.Square)
                hu = scr.tile([P, 512], bf16, name="hu", tag="hh")
                V.tensor_mul(hu, hT_sb[:, mh, csl], uT_sb[:, mh, csl])
                if mh == 0:
                    V.tensor_copy(y2a[:, csl], uu)
                    V.tensor_copy(xya[:, csl], hu)
                else:
                    V.tensor_add(y2a[:, csl], y2a[:, csl], uu)
                    V.tensor_add(xya[:, csl], xya[:, csl], hu)

            def mm2_stats(ch):
                csl = slice(ch * 512, (ch + 1) * 512)
                nc.tensor.matmul(stat_ps[ch][32:33, :], lhsT=ones,
                                 rhs=y2a[:, csl], start=True, stop=True,
                                 skip_group_check=True)
                nc.tensor.matmul(stat_ps[ch][64:65, :], lhsT=ones,
                                 rhs=xya[:, csl], start=True, stop=True,
                                 skip_group_check=True)
                hsl = slice(ch * 512, (ch + 1) * 512)
                stats_sb = scal.tile([P, 512], f32, name=f"stats_sb{ch}",
                                     tag="stats_sb")
                for i, r in enumerate((0, 32, 64)):
                    S.copy(stats_sb[r:r + 1, :], stat_ps[ch][r:r + 1, :])
                    nc.sync.dma_start(out=st_d[i, hsl],
                                      in_=stats_sb[r:r + 1, :])

            def zcomb(ch):
                csl = slice(ch * 512, (ch + 1) * 512)
                for kh in range(KH):
                    t1z = zscr.tile([P, 512], bf16, name="t1z", tag="zz")
                    V.tensor_mul(t1z, hT_sb[:, kh, csl], alpha_b[:, csl])
                    t2z = zscr.tile([P, 512], bf16, name="t2z", tag="zz")
                    V.tensor_mul(t2z, uT_sb[:, kh, csl], beta_b[:, csl])
                    V.tensor_add(uT_sb[:, kh, csl], t1z, t2z)

            # ---------- MM2 ch0 ----------
            with nc.named_scope("mm2a"):
                for mh in range(KH):
                    mm2_mh(0, mh)
                mm2_stats(0)
            with nc.named_scope("chain0"):
                scalar_chain(0)
            # ---------- MM2 ch1; scheduler interleaves zcomb0 ----------
            with nc.named_scope("mm2b"):
                for mh in range(18):
                    mm2_mh(1, mh)
                with nc.named_scope("zcomb0"):
                    zcomb(0)
                for mh in range(18, KH):
                    mm2_mh(1, mh)
                mm2_stats(1)
            with nc.named_scope("chain1"):
                scalar_chain(1)
        # ph1 psum pools (mm, stp) released here

        # ---------- MMo: out = z @ Wo; zcomb1 under mmo0 ------
        with ExitStack() as ph2:
            mmo = ph2.enter_context(tc.tile_pool(name="mmo", bufs=8,
                                                 space="PSUM"))

            def mmo_ch(ch):
                pso = [mmo.tile([P, 500], f32, name=f"pso{ch}_{i}",
                                tag="mmo") for i in range(8)]
                for kh in range(KH):
                    wot = wp.tile([P, OUT], bf16, name="wot", tag="w")
                    nc.sync.dma_start(out=wot, in_=wo_d[kh])
                    for i in range(4):
                        b = ch * 4 + i
                        for och in range(2):
                            nc.tensor.matmul(
                                pso[i * 2 + och],
                                lhsT=uT_sb[:, kh, b * P:(b + 1) * P],
                                rhs=wot[:, och * 500:(och + 1) * 500],
                                start=(kh == 0), stop=(kh == KH - 1))
                for i in range(4):
                    b = ch * 4 + i
                    for och in range(2):
                        osl = slice(och * 500, (och + 1) * 500)
                        ob = outp.tile([P, 500], f32, name="ob", tag="ob")
                        if och == 0:
                            S.copy(ob, pso[i * 2])
                        else:
                            V.tensor_copy(ob, pso[i * 2 + 1])
                        nc.sync.dma_start(
                            out=out_d[b * P:(b + 1) * P, osl], in_=ob)

            with nc.named_scope("zcomb1"):
                zcomb(1)
            with nc.named_scope("mmo0"):
                mmo_ch(0)
            with nc.named_scope("mmo1"):
                mmo_ch(1)

    nc.compile()
    return nc


def _get_nc(with_b1, with_b2):
    for k, v in _nc_cache:
        if k == (with_b1, with_b2):
            return v
    nc = _build(with_b1, with_b2)
    _nc_cache.append(((with_b1, with_b2), nc))
    return nc


def kernel(x, W1, b1, W2, b2, Wo, bo, cp_w1, cp_b1, cp_w2, cp_b2,
           _trace=False, _tmpdir=None):
    x = np.asarray(x, dtype=np.float32)
    with_b1 = bool(np.any(b1))
    with_b2 = bool(np.any(b2))
    nc = _get_nc(with_b1, with_b2)

    # w1r[mh, p, ki, q] = W1[ki*128+p, mh*128+q]
    w1_t = np.ascontiguousarray(
        np.asarray(W1, np.float32).reshape(KI, P, KH, P).transpose(2, 1, 0, 3)
    ).astype(BF)
    # w2r[mh, p, kp, j, q] = W2[(2*kp+j)*128+p, mh*128+q] * 256 in e4m3
    w2_t = np.ascontiguousarray(
        (np.asarray(W2, np.float32) * np.float32(W2_SCALE))
        .reshape(KP, 2, P, KH, P).transpose(3, 2, 0, 1, 4)
    ).astype(E4)
    wo_t = np.asarray(Wo, np.float32).reshape(KH, P, OUT).astype(BF)
    cpw1_t = np.ascontiguousarray(
        np.asarray(cp_w1, np.float32).T.reshape(KI, P, 16)).astype(BF)
    cpw2_t = np.asarray(cp_w2, np.float32).reshape(1, 16).T.astype(BF)
    cpw2_t = np.ascontiguousarray(cpw2_t)
    cpb1_t = np.asarray(cp_b1, np.float32).reshape(16, 1)
    cpb2_t = np.asarray(cp_b2, np.float32).reshape(1, 1)
    b1_t = np.ascontiguousarray(np.asarray(b1, np.float32).reshape(KH, P).T)
    b2_t = np.ascontiguousarray(np.asarray(b2, np.float32).reshape(KH, P).T)

    in_maps = []
    for c in range(N_CORES):
        shard = x[c * BL:(c + 1) * BL]
        xT = np.ascontiguousarray(shard.T).reshape(KI, P, BL).astype(BF)
        m = {"xT": xT, "w1": w1_t, "w2": w2_t, "wo": wo_t,
             "cpw1": cpw1_t, "cpw2": cpw2_t, "cpb1": cpb1_t, "cpb2": cpb2_t}
        if with_b1:
            m["b1"] = b1_t
        if with_b2:
            m["b2"] = b2_t
        in_maps.append(m)

    kw = {}
    if _trace:
        kw = dict(trace=True, tmpdir=_tmpdir or tempfile.mkdtemp(prefix="cdk_"))
    res = run_bass_kernel_spmd(nc, in_maps, list(range(N_CORES)), **kw)

    out = np.concatenate([res.results[c]["out"] for c in range(N_CORES)], axis=0)
    bo = np.asarray(bo, np.float32)
    if np.any(bo):
        out = out + bo
    if _trace:
        kernel._last_result = res
    return out


# revision 14
# speedup vs baseline: 1.0020x; 1.0020x over previous
"""Trainium2 Bass kernel for nn_ConservativeDynamicCurvatureMLP.

Data-parallel over 8 NeuronCores: batch (8192) sharded into 8 shards of
1024 rows; weights replicated.  The curvature scalar couples shards via a
single-scalar AllReduce.

Math (reference):
    h = tanh(x @ W1 + b1)
    u = sigmoid(h @ W2 + b2)
    c = clip(mean(MIN_C + (MAX_C-MIN_C) * sigmoid(relu(x@cp_w1.T+cp_b1)@cp_w2.T+cp_b2)), MIN_C, MAX_C)
    z = poincare_ball_layer(h, u, c, T)   ==  alpha(row)*h + beta(row)*u
    out = z @ Wo + bo

Performance structure (v2):
  * MM1 (x@W1) in bf16, feature-major, as before.
  * MM2 (h@W2) in fp8 e4m3 with DoubleRow perf mode: 2 k-slices per
    matmul -> 2x PE throughput.  h is cast bf16->e4m3 on DVE (direct cast,
    |h|<=1), W2 is host-prescaled by 256 into e4m3 (max |W2*256| ~ 28 << 240);
    the 1/256 is folded into the sigmoid activation scale.  Predicted
    end-to-end rel err ~1.3e-2 (gate 2e-2), simulated with exact RNE casts.
  * MM2 runs column-chunk-major (two 512-column halves): the per-half
    stats -> scalar-chain -> z-combine tail overlaps the other half's
    matmuls / the output projection, keeping the PE dense (no HAM
    re-throttle).
  * MMo (z@Wo) in bf16, och-wave structure, overlapped with the half-1
    z-combine.
"""

import tempfile
from contextlib import ExitStack

import numpy as np
import ml_dtypes

import concourse.bass as bass
import concourse.bacc as bacc
import concourse.mybir as mybir
import concourse.tile as tile
from concourse.bass_utils import run_bass_kernel_spmd

P = 128
N_CORES = 8
B_FULL = 8192
BL = B_FULL // N_CORES          # 1024 rows per core
IN = 3072
HID = 4096
OUT = 1000
KI = IN // P                    # 24
KH = HID // P                   # 32
KP = KH // 2                    # 16 DoubleRow k-pairs
MIN_C = 0.001 * 0.5
MAX_C = 0.001 * 2.0
T_CONST = 0.7
EPS = 1e-7
W2_SCALE = 256.0

dt = mybir.dt
AF = mybir.ActivationFunctionType
ALU = mybir.AluOpType
DR = mybir.MatmulPerfMode.DoubleRow
BF = ml_dtypes.bfloat16
E4 = ml_dtypes.float8_e4m3fn

_nc_cache = []


def _build(with_b1, with_b2):
    nc = bacc.Bacc("TRN2", target_bir_lowering=False, debug=False,
                   num_devices=N_CORES)

    xT_d = nc.dram_tensor("xT", [KI, P, BL], dt.bfloat16, kind="ExternalInput")
    # w1r[mh, p, ki, q] = W1[ki*128+p, mh*128+q]
    w1_d = nc.dram_tensor("w1", [KH, P, KI, P], dt.bfloat16, kind="ExternalInput")
    # w2r[mh, p, kp, j, q] = W2[(2*kp+j)*128+p, mh*128+q] * 256  (fp8 pairs)
    w2_d = nc.dram_tensor("w2", [KH, P, KP, 2, P], dt.float8e4,
                          kind="ExternalInput")
    wo_d = nc.dram_tensor("wo", [KH, P, OUT], dt.bfloat16, kind="ExternalInput")
    cpw1_d = nc.dram_tensor("cpw1", [KI, P, 16], dt.bfloat16, kind="ExternalInput")
    cpw2_d = nc.dram_tensor("cpw2", [16, 1], dt.bfloat16, kind="ExternalInput")
    cpb1_d = nc.dram_tensor("cpb1", [16, 1], dt.float32, kind="ExternalInput")
    cpb2_d = nc.dram_tensor("cpb2", [1, 1], dt.float32, kind="ExternalInput")
    b1_d = nc.dram_tensor("b1", [P, KH], dt.float32, kind="ExternalInput") if with_b1 else None
    b2_d = nc.dram_tensor("b2", [P, KH], dt.float32, kind="ExternalInput") if with_b2 else None
    out_d = nc.dram_tensor("out", [BL, OUT], dt.float32, kind="ExternalOutput")

    f32 = dt.float32
    bf16 = dt.bfloat16
    fp8 = dt.float8e4

    with tile.TileContext(nc) as tc, ExitStack() as ctx:
        const = ctx.enter_context(tc.tile_pool(name="const", bufs=1))
        big = ctx.enter_context(tc.tile_pool(name="big", bufs=1))
        htp = ctx.enter_context(tc.tile_pool(name="htp", bufs=1))
        hqp = ctx.enter_context(tc.tile_pool(name="hqp", bufs=1))
        wp = ctx.enter_context(tc.tile_pool(name="wp", bufs=2))
        scr = ctx.enter_context(tc.tile_pool(name="scr", bufs=3))
        zscr = ctx.enter_context(tc.tile_pool(name="zscr", bufs=2))
        sacc = ctx.enter_context(tc.tile_pool(name="sacc", bufs=2))
        abp = ctx.enter_context(tc.tile_pool(name="abp", bufs=1))
        scal = ctx.enter_context(tc.tile_pool(name="scal", bufs=1))
        outp = ctx.enter_context(tc.tile_pool(name="outp", bufs=2))
        cpp = ctx.enter_context(tc.tile_pool(name="cpp", bufs=1))
        dram = ctx.enter_context(tc.tile_pool(name="dram", bufs=1, space="DRAM"))

        V = nc.vector
        S = nc.scalar

        def sc(name, shape=(P, 8), dtype=f32):
            return scal.tile(list(shape), dtype, name=name, tag=name)

        # ---------- persistent activations (feature-major) ----------
        ones = const.tile([P, 1], f32, name="ones")
        nc.vector.memset(ones, 1.0)
        xT_sb = big.tile([P, KI, BL], bf16, name="xT_sb", tag="big",
                         padded_shape=[P, KH, BL])
        w1row0 = wp.tile([P, KI, P], bf16, name="w1row", tag="w")
        nc.sync.dma_start(out=w1row0, in_=w1_d[0])
        for a, b in ((0, 2), (2, 4), (4, 8), (8, 12), (12, 16), (16, 20),
                     (20, 24)):
            nc.gpsimd.dma_start(
                out=xT_sb[:, a:b, :],
                in_=xT_d[a:b].rearrange("k p b -> p k b"))
        hT_sb = htp.tile([P, KH, BL], bf16, name="hT_sb")
        hq_sb = hqp.tile([P, KP, 2, BL], fp8, name="hq_sb")
        if with_b1:
            b1_sb = const.tile([P, KH], f32, name="b1_sb")
            nc.sync.dma_start(out=b1_sb, in_=b1_d[:, :])
        if with_b2:
            b2_sb = const.tile([P, KH], f32, name="b2_sb")
            nc.sync.dma_start(out=b2_sb, in_=b2_d[:, :])

        st_d = dram.tile([3, BL], f32, name="st_d")
        ab_d = dram.tile([2, BL], bf16, name="ab_d")
        alpha_b = abp.tile([P, BL], bf16, name="alpha_b")
        beta_b = abp.tile([P, BL], bf16, name="beta_b")

        with ExitStack() as ph1:
            mm = ph1.enter_context(tc.tile_pool(name="mm", bufs=2, space="PSUM"))
            stp = ph1.enter_context(tc.tile_pool(name="stp", bufs=2, space="PSUM"))
            # per-half stat psums; rows: x2 @ 0, y2 @ 32, xy @ 64
            stat_ps = [stp.tile([P, 512], f32, name=f"stat_ps{ch}",
                                tag="stat") for ch in range(2)]

            # ---------- MM1: hT = tanh(W1.T @ xT), hq cast, x2 stats ------
            x2a = sacc.tile([P, BL], f32, name="x2a", tag="sacc")
            with nc.named_scope("mm1"):
                for mh in range(KH):
                    ps = mm.tile([P, BL], f32, name="ps", tag="mm")
                    if mh == 0:
                        w1row = w1row0
                    else:
                        w1row = wp.tile([P, KI, P], bf16, name="w1row",
                                        tag="w")
                        nc.sync.dma_start(out=w1row, in_=w1_d[mh])
                    for ki in range(KI):
                        nc.tensor.matmul(ps[:, 0:512], lhsT=w1row[:, ki, :],
                                         rhs=xT_sb[:, ki, 0:512],
                                         start=(ki == 0), stop=(ki == KI - 1))
                        nc.tensor.matmul(ps[:, 512:BL], lhsT=w1row[:, ki, :],
                                         rhs=xT_sb[:, ki, 512:BL],
                                         start=(ki == 0), stop=(ki == KI - 1))
                    if with_b1:
                        S.activation(hT_sb[:, mh, :], ps, AF.Tanh,
                                     bias=b1_sb[:, mh:mh + 1])
                    else:
                        S.activation(hT_sb[:, mh, :], ps, AF.Tanh)
                    V.tensor_copy(hq_sb[:, mh // 2, mh % 2, :],
                                  hT_sb[:, mh, :])
                    hh = scr.tile([P, BL], bf16, name="hh", tag="hh")
                    S.activation(hh, hT_sb[:, mh, :], AF.Square)
                    if mh == 0:
                        V.tensor_copy(x2a, hh)
                    else:
                        V.tensor_add(x2a, x2a, hh)
                for ch in range(2):
                    sl = slice(ch * 512, (ch + 1) * 512)
                    nc.tensor.matmul(stat_ps[ch][0:1, :], lhsT=ones,
                                     rhs=x2a[:, sl], start=True, stop=True,
                                     skip_group_check=True)

            # ---------- curvature predictor + AllReduce ----------
            with nc.named_scope("cp"):
                cpw1_sb = const.tile([P, KI, 16], bf16, name="cpw1_sb")
                nc.sync.dma_start(out=cpw1_sb,
                                  in_=cpw1_d.rearrange("k p q -> p k q"))
                cpw2_sb = const.tile([16, 1], bf16, name="cpw2_sb")
                nc.sync.dma_start(out=cpw2_sb, in_=cpw2_d[:, :])
                cpb1_sb = const.tile([16, 1], f32, name="cpb1_sb")
                nc.sync.dma_start(out=cpb1_sb, in_=cpb1_d[:, :])
                cpb2_sb = const.tile([1, 1], f32, name="cpb2_sb")
                nc.sync.dma_start(out=cpb2_sb, in_=cpb2_d[:, :])
                cph_sb = cpp.tile([16, BL], fp8, name="cph_sb")
                for ch in range(2):
                    cps = mm.tile([16, 512], f32, name="cps", tag="mm")
                    for ki in range(KI):
                        nc.tensor.matmul(
                            cps, lhsT=cpw1_sb[:, ki, :],
                            rhs=xT_sb[:, ki, ch * 512:(ch + 1) * 512],
                            start=(ki == 0), stop=(ki == KI - 1))
                    S.activation(cph_sb[:, ch * 512:(ch + 1) * 512], cps,
                                 AF.Relu, bias=cpb1_sb)
                sparts = []
                for ch in range(2):
                    c2p = mm.tile([1, 512], f32, name="c2p", tag="mm")
                    nc.tensor.matmul(c2p, lhsT=cpw2_sb,
                                     rhs=cph_sb[:16, ch * 512:(ch + 1) * 512],
                                     start=True, stop=True)
                    cpw = cpp.tile([1, 512], bf16, name="cpw", tag="cpw")
                    spart = cpp.tile([1, 1], f32, name=f"spart{ch}",
                                     tag=f"spart{ch}")
                    S.activation(cpw, c2p, AF.Sigmoid, bias=cpb2_sb,
                                 accum_out=spart)
                    sparts.append(spart)
                s_loc = cpp.tile([1, 1], f32, name="s_loc")
                V.tensor_add(s_loc, sparts[0], sparts[1])
                cin = dram.tile([1, 1], f32, name="cin")
                cout = dram.tile([1, 1], f32, name="cout")
                nc.sync.dma_start(out=cin, in_=s_loc)
                nc.gpsimd.collective_compute(
                    "AllReduce", ALU.add,
                    replica_groups=[list(range(N_CORES))],
                    ins=[cin.opt()], outs=[cout.opt()])
                s_b = sc("s_b", (P, 1))
                nc.gpsimd.dma_start(out=s_b, in_=cout.to_broadcast([P, 1]))
                c_b = sc("c_b", (P, 1))
                V.tensor_scalar(out=c_b, in0=s_b,
                                scalar1=(MAX_C - MIN_C) / B_FULL,
                                scalar2=MIN_C, op0=ALU.mult, op1=ALU.add)
                V.tensor_scalar_min(out=c_b, in0=c_b, scalar1=MAX_C)
                V.tensor_scalar_max(out=c_b, in0=c_b, scalar1=MIN_C)
                negc_b = sc("negc_b", (P, 1))
                V.tensor_scalar_mul(out=negc_b, in0=c_b, scalar1=-1.0)
                twoc_b = sc("twoc_b", (P, 1))
                V.tensor_scalar_mul(out=twoc_b, in0=c_b, scalar1=2.0)
                neg2c_b = sc("neg2c_b", (P, 1))
                V.tensor_scalar_mul(out=neg2c_b, in0=c_b, scalar1=-2.0)
                c2_b = sc("c2_b", (P, 1))
                V.tensor_mul(c2_b, c_b, c_b)

        # still inside ph1 scope vars; reopened below for MM2
            # ---------- per-row scalar chain (batch-major [128, 4]) -------
            y2a = sacc.tile([P, BL], f32, name="y2a", tag="sacc")
            xya = sacc.tile([P, BL], f32, name="xya", tag="sacc")
            uT_sb = big.tile([P, KH, BL], bf16, name="uT_sb", tag="big")

            def scalar_chain(ch):
                hsl = slice(ch * 512, (ch + 1) * 512)

                def sch(name):
                    return sc(f"{name}_{ch}", (P, 4))

                x2 = sch("x2")
                y2 = sch("y2")
                xy = sch("xy")
                for i, t in enumerate((x2, y2, xy)):
                    nc.scalar.dma_start(
                        out=t, in_=st_d[i, hsl].rearrange("(j p) -> p j", p=P))
                w = sch("w")
                V.scalar_tensor_tensor(out=w, in0=xy, scalar=-2.0, in1=y2,
                                       op0=ALU.mult, op1=ALU.add)
                A1 = sch("A1")
                V.tensor_scalar(out=A1, in0=w, scalar1=c_b, scalar2=1.0,
                                op0=ALU.mult, op1=ALU.add)
                A2 = sch("A2")
                V.tensor_scalar(out=A2, in0=x2, scalar1=negc_b, scalar2=1.0,
                                op0=ALU.mult, op1=ALU.add)
                p1 = sch("p1")
                V.tensor_mul(p1, x2, y2)
                den = sch("den")
                V.tensor_scalar(out=den, in0=p1, scalar1=c2_b, scalar2=1.0,
                                op0=ALU.mult, op1=ALU.add)
                V.scalar_tensor_tensor(out=den, in0=xy, scalar=neg2c_b, in1=den,
                                       op0=ALU.mult, op1=ALU.add)
                V.tensor_scalar_add(out=den, in0=den, scalar1=EPS)
                D = sch("D")
                V.reciprocal(D, den)
                t1 = sch("t1")
                V.tensor_mul(t1, A1, A1)
                V.tensor_mul(t1, t1, x2)
                t2 = sch("t2")
                V.tensor_mul(t2, A1, A2)
                V.tensor_mul(t2, t2, xy)
                t3 = sch("t3")
                V.tensor_mul(t3, A2, A2)
                V.tensor_mul(t3, t3, y2)
                na2 = sch("na2")
                V.scalar_tensor_tensor(out=na2, in0=t2, scalar=-2.0, in1=t1,
                                       op0=ALU.mult, op1=ALU.add)
                V.tensor_add(na2, na2, t3)
                dsq = sch("dsq")
                V.tensor_mul(dsq, D, D)
                V.tensor_mul(na2, na2, dsq)
                # tm = tanh(T*artanh(sqrt(s)))/sqrt(s) with s = c*na2 is
                # analytic in s; deg-12 poly on s in [0.5, 0.996] (max rel
                # err 5.7e-4 over the c-envelope, 4.5e-5 in the actual
                # band) -- keeps the whole chain on DVE (no ACT table
                # thrash against MM2's sigmoids).
                s_t = sch("s_t")
                V.tensor_scalar(out=s_t, in0=na2, scalar1=c_b, scalar2=None,
                                op0=ALU.mult)
                V.tensor_scalar_min(out=s_t, in0=s_t, scalar1=0.996)
                V.tensor_scalar_max(out=s_t, in0=s_t, scalar1=0.5)
                V.tensor_scalar_add(out=s_t, in0=s_t, scalar1=-0.75)
                TM_POLY = (2227824.6408410813, 448871.7528227819,
                           -312401.2221121575, -56799.3889483669,
                           17050.8363088851, 2766.4915063889557,
                           -404.5213056958804, -51.417097735340924,
                           6.522608450512562, 1.2576027346248937,
                           0.3868927385367392, 0.32011781072746887,
                           0.839226248286217)
                tm = sch("tm")
                tmw = sch("tmw")
                V.tensor_scalar(out=tm, in0=s_t, scalar1=TM_POLY[0],
                                scalar2=TM_POLY[1], op0=ALU.mult, op1=ALU.add)
                for cf_ in TM_POLY[2:]:
                    V.tensor_mul(tmw, tm, s_t)
                    V.tensor_scalar_add(out=tm, in0=tmw, scalar1=cf_)
                s1_ = sch("s1_")
                V.tensor_mul(s1_, A1, x2)
                s2_ = sch("s2_")
                V.tensor_mul(s2_, A2, xy)
                ha = sch("ha")
                V.tensor_sub(ha, s2_, s1_)
                V.tensor_mul(ha, ha, D)
                hm = sch("hm")
                V.tensor_mul(hm, tm, ha)
                tsq = sch("tsq")
                V.tensor_mul(tsq, tm, tm)
                m2 = sch("m2")
                V.tensor_mul(m2, tsq, na2)
                w2s = sch("w2s")
                V.scalar_tensor_tensor(out=w2s, in0=hm, scalar=2.0, in1=m2,
                                       op0=ALU.mult, op1=ALU.add)
                B1 = sch("B1")
                V.tensor_scalar(out=B1, in0=w2s, scalar1=c_b, scalar2=1.0,
                                op0=ALU.mult, op1=ALU.add)
                p2 = sch("p2")
                V.tensor_mul(p2, x2, m2)
                den2 = sch("den2")
                V.tensor_scalar(out=den2, in0=p2, scalar1=c2_b, scalar2=1.0,
                                op0=ALU.mult, op1=ALU.add)
                V.scalar_tensor_tensor(out=den2, in0=hm, scalar=twoc_b, in1=den2,
                                       op0=ALU.mult, op1=ALU.add)
                V.tensor_scalar_add(out=den2, in0=den2, scalar1=EPS)
                D2 = sch("D2")
                V.reciprocal(D2, den2)
                g = sch("g")
                V.tensor_mul(g, A2, tm)
                V.tensor_mul(g, g, D)
                w3 = sch("w3")
                V.tensor_mul(w3, g, A1)
                V.tensor_sub(w3, B1, w3)
                alpha_bm = sc(f"alpha_bm_{ch}", (P, 4), bf16)
                V.tensor_mul(alpha_bm, w3, D2)
                w4 = sch("w4")
                V.tensor_mul(w4, g, A2)
                beta_bm = sc(f"beta_bm_{ch}", (P, 4), bf16)
                V.tensor_mul(beta_bm, w4, D2)
                # store + broadcast on the vector queue: zero cross-engine
                # wake latency into the zcomb consumers, and keeps the sync
                # queue free for weight streaming
                nc.scalar.dma_start(
                    out=ab_d[0, hsl].rearrange("(j p) -> p j", p=P),
                    in_=alpha_bm)
                nc.scalar.dma_start(
                    out=ab_d[1, hsl].rearrange("(j p) -> p j", p=P),
                    in_=beta_bm)
                nc.scalar.dma_start(out=alpha_b[:, hsl],
                                    in_=ab_d[0:1, hsl].to_broadcast([P, 512]))
                nc.scalar.dma_start(out=beta_b[:, hsl],
                                    in_=ab_d[1:2, hsl].to_broadcast([P, 512]))

            def mm2_mh(ch, mh):
                csl = slice(ch * 512, (ch + 1) * 512)
                ps = mm.tile([P, 512], f32, name="ps2", tag="mm")
                w2row = wp.tile([P, KP, 2, P], fp8, name="w2row", tag="w")
                nc.sync.dma_start(out=w2row, in_=w2_d[mh])
                for kp in range(KP):
                    nc.tensor.matmul(ps, lhsT=w2row[:, kp],
                                     rhs=hq_sb[:, kp, :, csl],
                                     start=(kp == 0), stop=(kp == KP - 1),
                                     perf_mode=DR)
                if with_b2:
                    S.activation(uT_sb[:, mh, csl], ps, AF.Sigmoid,
                                 bias=b2_sb[:, mh:mh + 1],
                                 scale=1.0 / W2_SCALE)
                else:
                    S.activation(uT_sb[:, mh, csl], ps, AF.Sigmoid,
                                 scale=1.0 / W2_SCALE)
                uu = scr.tile([P, 512], bf16, name="uu", tag="hh")
                S.activation(uu, uT_sb[:, mh, csl], AF.Square)
                hu = scr.tile([P, 512], bf16, name="hu", tag="hh")
                V.tensor_mul(hu, hT_sb[:, mh, csl], uT_sb[:, mh, csl])
                if mh == 0:
                    V.tensor_copy(y2a[:, csl], uu)
                    V.tensor_copy(xya[:, csl], hu)
                else:
                    V.tensor_add(y2a[:, csl], y2a[:, csl], uu)
                    V.tensor_add(xya[:, csl], xya[:, csl], hu)

            def mm2_stats(ch):
                csl = slice(ch * 512, (ch + 1) * 512)
                nc.tensor.matmul(stat_ps[ch][32:33, :], lhsT=ones,
                                 rhs=y2a[:, csl], start=True, stop=True,
                                 skip_group_check=True)
                nc.tensor.matmul(stat_ps[ch][64:65, :], lhsT=ones,
                                 rhs=xya[:, csl], start=True, stop=True,
                                 skip_group_check=True)
                hsl = slice(ch * 512, (ch + 1) * 512)
                stats_sb = scal.tile([P, 512], f32, name=f"stats_sb{ch}",
                                     tag="stats_sb")
                for i, r in enumerate((0, 32, 64)):
                    S.copy(stats_sb[r:r + 1, :], stat_ps[ch][r:r + 1, :])
                    nc.sync.dma_start(out=st_d[i, hsl],
                                      in_=stats_sb[r:r + 1, :])

            def zcomb(ch):
                csl = slice(ch * 512, (ch + 1) * 512)
                for kh in range(KH):
                    t1z = zscr.tile([P, 512], bf16, name="t1z", tag="zz")
                    V.tensor_mul(t1z, hT_sb[:, kh, csl], alpha_b[:, csl])
                    t2z = zscr.tile([P, 512], bf16, name="t2z", tag="zz")
                    V.tensor_mul(t2z, uT_sb[:, kh, csl], beta_b[:, csl])
                    V.tensor_add(uT_sb[:, kh, csl], t1z, t2z)

            # ---------- MM2 ch0 ----------
            with nc.named_scope("mm2a"):
                for mh in range(KH):
                    mm2_mh(0, mh)
                mm2_stats(0)
            # high_priority: schedule the chain compactly as soon as its
            # deps allow -- its DMA completions gate later same-semaphore
            # weight DMAs (shared-semaphore head-of-line blocking)
            with nc.named_scope("chain0"), tc.high_priority():
                scalar_chain(0)
            # ---------- MM2 ch1; scheduler interleaves zcomb0 ----------
            with nc.named_scope("mm2b"):
                for mh in range(18):
                    mm2_mh(1, mh)
                with nc.named_scope("zcomb0"):
                    zcomb(0)
                for mh in range(18, KH):
                    mm2_mh(1, mh)
                mm2_stats(1)
        # ph1 psum pools (mm, stp) released here

        # ---------- MMo: out = z @ Wo; zcomb1 under mmo0 ------
        with ExitStack() as ph2:
            mmo = ph2.enter_context(tc.tile_pool(name="mmo", bufs=8,
                                                 space="PSUM"))

            def mmo_ch(ch):
                pso = [mmo.tile([P, 500], f32, name=f"pso{ch}_{i}",
                                tag="mmo") for i in range(8)]
                for kh in range(KH):
                    wot = wp.tile([P, OUT], bf16, name="wot", tag="w")
                    nc.sync.dma_start(out=wot, in_=wo_d[kh])
                    for i in range(4):
                        b = ch * 4 + i
                        for och in range(2):
                            nc.tensor.matmul(
                                pso[i * 2 + och],
                                lhsT=uT_sb[:, kh, b * P:(b + 1) * P],
                                rhs=wot[:, och * 500:(och + 1) * 500],
                                start=(kh == 0), stop=(kh == KH - 1))
                for i in range(4):
                    b = ch * 4 + i
                    for och in range(2):
                        osl = slice(och * 500, (och + 1) * 500)
                        ob = outp.tile([P, 500], f32, name="ob", tag="ob")
                        if och == 0:
                            S.copy(ob, pso[i * 2])
                        else:
                            V.tensor_copy(ob, pso[i * 2 + 1])
                        nc.sync.dma_start(
                            out=out_d[b * P:(b + 1) * P, osl], in_=ob)

            # mmo0's wot DMAs emitted BEFORE chain1's so the shared DMA
            # semaphores don't serialize the weight stream behind the chain
            with nc.named_scope("mmo0"):
                mmo_ch(0)
            with nc.named_scope("chain1"), tc.high_priority():
                scalar_chain(1)
            with nc.named_scope("zcomb1"):
                zcomb(1)
            with nc.named_scope("mmo1"):
                mmo_ch(1)

    nc.compile()
    return nc


def _get_nc(with_b1, with_b2):
    for k, v in _nc_cache:
        if k == (with_b1, with_b2):
            return v
    nc = _build(with_b1, with_b2)
    _nc_cache.append(((with_b1, with_b2), nc))
    return nc


def kernel(x, W1, b1, W2, b2, Wo, bo, cp_w1, cp_b1, cp_w2, cp_b2,
           _trace=False, _tmpdir=None):
    x = np.asarray(x, dtype=np.float32)
    with_b1 = bool(np.any(b1))
    with_b2 = bool(np.any(b2))
    nc = _get_nc(with_b1, with_b2)

    # w1r[mh, p, ki, q] = W1[ki*128+p, mh*128+q]
    w1_t = np.ascontiguousarray(
        np.asarray(W1, np.float32).reshape(KI, P, KH, P).transpose(2, 1, 0, 3)
    ).astype(BF)
    # w2r[mh, p, kp, j, q] = W2[(2*kp+j)*128+p, mh*128+q] * 256 in e4m3
    w2_t = np.ascontiguousarray(
        (np.asarray(W2, np.float32) * np.float32(W2_SCALE))
        .reshape(KP, 2, P, KH, P).transpose(3, 2, 0, 1, 4)
    ).astype(E4)
    wo_t = np.asarray(Wo, np.float32).reshape(KH, P, OUT).astype(BF)
    cpw1_t = np.ascontiguousarray(
        np.asarray(cp_w1, np.float32).T.reshape(KI, P, 16)).astype(BF)
    cpw2_t = np.asarray(cp_w2, np.float32).reshape(1, 16).T.astype(BF)
    cpw2_t = np.ascontiguousarray(cpw2_t)
    cpb1_t = np.asarray(cp_b1, np.float32).reshape(16, 1)
    cpb2_t = np.asarray(cp_b2, np.float32).reshape(1, 1)
    b1_t = np.ascontiguousarray(np.asarray(b1, np.float32).reshape(KH, P).T)
    b2_t = np.ascontiguousarray(np.asarray(b2, np.float32).reshape(KH, P).T)

    in_maps = []
    for c in range(N_CORES):
        shard = x[c * BL:(c + 1) * BL]
        xT = np.ascontiguousarray(shard.T).reshape(KI, P, BL).astype(BF)
        m = {"xT": xT, "w1": w1_t, "w2": w2_t, "wo": wo_t,
             "cpw1": cpw1_t, "cpw2": cpw2_t, "cpb1": cpb1_t, "cpb2": cpb2_t}
        if with_b1:
            m["b1"] = b1_t
        if with_b2:
            m["b2"] = b2_t
        in_maps.append(m)

    kw = {}
    if _trace:
        kw = dict(trace=True, tmpdir=_tmpdir or tempfile.mkdtemp(prefix="cdk_"))
    res = run_bass_kernel_spmd(nc, in_maps, list(range(N_CORES)), **kw)

    out = np.concatenate([res.results[c]["out"] for c in range(N_CORES)], axis=0)
    bo = np.asarray(bo, np.float32)
    if np.any(bo):
        out = out + bo
    if _trace:
        kernel._last_result = res
    return out


# revision 29
# speedup vs baseline: 1.1455x; 1.1431x over previous
"""Trainium2 Bass kernel for nn_ConservativeDynamicCurvatureMLP.

Data-parallel over 8 NeuronCores: batch (8192) sharded into 8 shards of
1024 rows; weights replicated.  The curvature scalar couples shards via a
single-scalar AllReduce.

Math (reference):
    h = tanh(x @ W1 + b1)
    u = sigmoid(h @ W2 + b2)
    c = clip(mean(MIN_C + (MAX_C-MIN_C) * sigmoid(relu(x@cp_w1.T+cp_b1)@cp_w2.T+cp_b2)), MIN_C, MAX_C)
    z = poincare_ball_layer(h, u, c, T)   ==  alpha(row)*h + beta(row)*u
    out = z @ Wo + bo

Performance structure (measured 862us vs 1177us bf16 baseline; rel err
1.50e-2 vs the 2e-2 gate, validated against an exact-RNE numpy model):

  * MM2 (h@W2) entirely in fp8 e4m3 with DoubleRow perf mode (2 k-slices
    per matmul -> 2x PE column rate, probe-verified 259ns per K=256,N=512
    matmul).  h is cast bf16->e4m3 on DVE (direct cast, |h|<=1), W2 is
    host-prescaled by 256 (max |W2*256| ~ 28 << 240); 1/256 folds into
    the sigmoid activation scale.
  * MM1 (x@W1): last 6 of 24 k-slices in fp8 DoubleRow, sharing the psum
    accumulation with the bf16 slices via the scale split x/16 * (W1*16).
  * MM2 runs column-half-major: each half's stats -> scalar chain ->
    z-combine overlaps the other half's matmuls / the output projection.
  * The scalar chain is DVE-only: tanh(T*artanh(sqrt(s)))/sqrt(s) is
    evaluated as a deg-12 polynomial in s = c*||a||^2 (no ACT table
    thrash against MM2's psum-evicting sigmoids).
  * All chain data movement is on the PE (zero DMAs): stats rows ->
    batch-major via K=1 matmuls against ones; alpha/beta -> feature-major
    broadcast via per-column transposes into diagonal blocks + one
    ones-matmul row-sum.  DMA-based bounces serialize behind the paced
    weight streams (the framework orders same-semaphore DMAs strictly).
  * MMo (z@Wo) in bf16 on 8 psum banks, full-width Wo tiles from a
    3-deep pool; output staged bf16 and cast to f32 by gpsimd DMA.
"""

import tempfile
from contextlib import ExitStack

import numpy as np
import ml_dtypes

import concourse.bass as bass
import concourse.bacc as bacc
import concourse.mybir as mybir
import concourse.tile as tile
from concourse.bass_utils import run_bass_kernel_spmd
from concourse.masks import make_identity

P = 128
N_CORES = 8
B_FULL = 8192
BL = B_FULL // N_CORES          # 1024 rows per core
IN = 3072
HID = 4096
OUT = 1000
KI = IN // P                    # 24
KB = 18                         # bf16 k-slices of MM1
KQ = (KI - KB) // 2             # 3 fp8 DoubleRow k-pairs of MM1
KH = HID // P                   # 32
KP = KH // 2                    # 16 DoubleRow k-pairs
X_SCALE = 1.0 / 16.0
W1_SCALE = 16.0
MIN_C = 0.001 * 0.5
MAX_C = 0.001 * 2.0
T_CONST = 0.7
EPS = 1e-7
W2_SCALE = 256.0

dt = mybir.dt
AF = mybir.ActivationFunctionType
ALU = mybir.AluOpType
DR = mybir.MatmulPerfMode.DoubleRow
BF = ml_dtypes.bfloat16
E4 = ml_dtypes.float8_e4m3fn

_nc_cache = []


def _build(with_b1, with_b2):
    nc = bacc.Bacc("TRN2", target_bir_lowering=False, debug=False,
                   num_devices=N_CORES)

    xT_d = nc.dram_tensor("xT", [KB, P, BL], dt.bfloat16, kind="ExternalInput")
    # xq[kp, p, j, b] = x[b, (KB+2kp+j)*128+p] / 16  (fp8 pairs)
    xq_d = nc.dram_tensor("xq", [KQ, P, 2, BL], dt.float8e4,
                          kind="ExternalInput")
    # w1r[mh, p, ki, q] = W1[ki*128+p, mh*128+q]  (bf16 slices)
    w1_d = nc.dram_tensor("w1", [KH, P, KB, P], dt.bfloat16, kind="ExternalInput")
    # w1q[mh, p, kp, j, q] = W1[(KB+2kp+j)*128+p, mh*128+q] * 16
    w1q_d = nc.dram_tensor("w1q", [KH, P, KQ, 2, P], dt.float8e4,
                           kind="ExternalInput")
    # w2r[mh, p, kp, j, q] = W2[(2*kp+j)*128+p, mh*128+q] * 256  (fp8 pairs)
    w2_d = nc.dram_tensor("w2", [KH, P, KP, 2, P], dt.float8e4,
                          kind="ExternalInput")
    wo_d = nc.dram_tensor("wo", [KH, P, OUT], dt.bfloat16, kind="ExternalInput")
    cpw1_d = nc.dram_tensor("cpw1", [KB, P, 16], dt.bfloat16, kind="ExternalInput")
    cpw1q_d = nc.dram_tensor("cpw1q", [KQ, P, 2, 16], dt.float8e4,
                             kind="ExternalInput")
    cpw2_d = nc.dram_tensor("cpw2", [16, 1], dt.bfloat16, kind="ExternalInput")
    cpb1_d = nc.dram_tensor("cpb1", [16, 1], dt.float32, kind="ExternalInput")
    cpb2_d = nc.dram_tensor("cpb2", [1, 1], dt.float32, kind="ExternalInput")
    b1_d = nc.dram_tensor("b1", [P, KH], dt.float32, kind="ExternalInput") if with_b1 else None
    b2_d = nc.dram_tensor("b2", [P, KH], dt.float32, kind="ExternalInput") if with_b2 else None
    out_d = nc.dram_tensor("out", [BL, OUT], dt.float32, kind="ExternalOutput")

    f32 = dt.float32
    bf16 = dt.bfloat16
    fp8 = dt.float8e4

    with tile.TileContext(nc) as tc, ExitStack() as ctx:
        const = ctx.enter_context(tc.tile_pool(name="const", bufs=1))
        big = ctx.enter_context(tc.tile_pool(name="big", bufs=1))
        htp = ctx.enter_context(tc.tile_pool(name="htp", bufs=1))
        hqp = ctx.enter_context(tc.tile_pool(name="hqp", bufs=1))
        wp = ctx.enter_context(tc.tile_pool(name="wp", bufs=2))
        wop = ctx.enter_context(tc.tile_pool(name="wop", bufs=3))
        scr = ctx.enter_context(tc.tile_pool(name="scr", bufs=2))
        zscr = ctx.enter_context(tc.tile_pool(name="zscr", bufs=2))
        sacc = ctx.enter_context(tc.tile_pool(name="sacc", bufs=2))
        abp = ctx.enter_context(tc.tile_pool(name="abp", bufs=1))
        scal = ctx.enter_context(tc.tile_pool(name="scal", bufs=1))
        outp = ctx.enter_context(tc.tile_pool(name="outp", bufs=2))
        dram = ctx.enter_context(tc.tile_pool(name="dram", bufs=1, space="DRAM"))

        V = nc.vector
        S = nc.scalar

        def sc(name, shape=(P, 8), dtype=f32):
            return scal.tile(list(shape), dtype, name=name, tag=name)

        # ---------- persistent activations (feature-major) ----------
        ones = const.tile([P, 1], f32, name="ones")
        nc.vector.memset(ones, 1.0)
        ones_row = const.tile([P, 128], bf16, name="ones_row")
        nc.vector.memset(ones_row, 1.0)
        ident = const.tile([P, P], bf16, name="ident")
        make_identity(nc, ident)
        xT_sb = big.tile([P, KB, BL], bf16, name="xT_sb", tag="big",
                         padded_shape=[P, KH, BL])
        xq_sb = hqp.tile([P, KQ, 2, BL], fp8, name="xq_sb", tag="xq")
        w1row0 = wp.tile([P, KB, P], bf16, name="w1row", tag="w")
        nc.sync.dma_start(out=w1row0, in_=w1_d[0])
        w1rowq0 = wp.tile([P, KQ, 2, P], fp8, name="w1rowq", tag="wq")
        nc.sync.dma_start(out=w1rowq0, in_=w1q_d[0])
        nc.gpsimd.dma_start(out=xq_sb,
                            in_=xq_d.rearrange("k p j b -> p k j b"))
        for a, b in ((0, 2), (2, 4), (4, 8), (8, 12), (12, 16), (16, 18)):
            nc.gpsimd.dma_start(
                out=xT_sb[:, a:b, :],
                in_=xT_d[a:b].rearrange("k p b -> p k b"))
        hT_sb = htp.tile([P, KH, BL], bf16, name="hT_sb")
        hq_sb = hqp.tile([P, KP, 2, BL], fp8, name="hq_sb")
        if with_b1:
            b1_sb = const.tile([P, KH], f32, name="b1_sb")
            nc.sync.dma_start(out=b1_sb, in_=b1_d[:, :])
        if with_b2:
            b2_sb = const.tile([P, KH], f32, name="b2_sb")
            nc.sync.dma_start(out=b2_sb, in_=b2_d[:, :])

        alpha_b = abp.tile([P, BL], bf16, name="alpha_b")
        beta_b = abp.tile([P, BL], bf16, name="beta_b")

        with ExitStack() as ph1:
            mm = ph1.enter_context(tc.tile_pool(name="mm", bufs=2, space="PSUM"))
            stp = ph1.enter_context(tc.tile_pool(name="stp", bufs=2, space="PSUM"))
            bcp = ph1.enter_context(tc.tile_pool(name="bcp", bufs=1, space="PSUM"))
            # per-half stat psums; rows: x2 @ 0, y2 @ 32, xy @ 64
            stat_ps = [stp.tile([P, 512], f32, name=f"stat_ps{ch}",
                                tag="stat") for ch in range(2)]

            # ---------- MM1: hT = tanh(W1.T @ xT), hq cast, x2 stats ------
            x2a = sacc.tile([P, BL], f32, name="x2a", tag="sacc")
            with nc.named_scope("mm1"):
                for mh in range(KH):
                    ps = mm.tile([P, BL], f32, name="ps", tag="mm")
                    if mh == 0:
                        w1row = w1row0
                        w1rowq = w1rowq0
                    else:
                        w1row = wp.tile([P, KB, P], bf16, name="w1row",
                                        tag="w")
                        nc.sync.dma_start(out=w1row, in_=w1_d[mh])
                        w1rowq = wp.tile([P, KQ, 2, P], fp8, name="w1rowq",
                                         tag="wq")
                        nc.sync.dma_start(out=w1rowq, in_=w1q_d[mh])
                    for ki in range(KB):
                        nc.tensor.matmul(ps[:, 0:512], lhsT=w1row[:, ki, :],
                                         rhs=xT_sb[:, ki, 0:512],
                                         start=(ki == 0), stop=False)
                        nc.tensor.matmul(ps[:, 512:BL], lhsT=w1row[:, ki, :],
                                         rhs=xT_sb[:, ki, 512:BL],
                                         start=(ki == 0), stop=False)
                    for kp in range(KQ):
                        nc.tensor.matmul(ps[:, 0:512], lhsT=w1rowq[:, kp],
                                         rhs=xq_sb[:, kp, :, 0:512],
                                         start=False, stop=(kp == KQ - 1),
                                         perf_mode=DR)
                        nc.tensor.matmul(ps[:, 512:BL], lhsT=w1rowq[:, kp],
                                         rhs=xq_sb[:, kp, :, 512:BL],
                                         start=False, stop=(kp == KQ - 1),
                                         perf_mode=DR)
                    if with_b1:
                        S.activation(hT_sb[:, mh, :], ps, AF.Tanh,
                                     bias=b1_sb[:, mh:mh + 1])
                    else:
                        S.activation(hT_sb[:, mh, :], ps, AF.Tanh)
                    V.tensor_copy(hq_sb[:, mh // 2, mh % 2, :],
                                  hT_sb[:, mh, :])
                    for hc in range(2):
                        hsl_ = slice(hc * 512, (hc + 1) * 512)
                        hh = scr.tile([P, 512], bf16, name="hh", tag="hh")
                        S.activation(hh, hT_sb[:, mh, hsl_], AF.Square)
                        if mh == 0:
                            V.tensor_copy(x2a[:, hsl_], hh)
                        else:
                            V.tensor_add(x2a[:, hsl_], x2a[:, hsl_], hh)
                for ch in range(2):
                    sl = slice(ch * 512, (ch + 1) * 512)
                    nc.tensor.matmul(stat_ps[ch][0:1, :], lhsT=ones,
                                     rhs=x2a[:, sl], start=True, stop=True,
                                     skip_group_check=True)

            # ---------- curvature predictor + AllReduce ----------
            with nc.named_scope("cp"):
                cpw1_sb = const.tile([P, KB, 16], bf16, name="cpw1_sb")
                nc.sync.dma_start(out=cpw1_sb,
                                  in_=cpw1_d.rearrange("k p q -> p k q"))
                cpw1q_sb = const.tile([P, KQ, 2, 16], fp8, name="cpw1q_sb")
                nc.sync.dma_start(out=cpw1q_sb,
                                  in_=cpw1q_d.rearrange("k p j q -> p k j q"))
                cpw2_sb = const.tile([16, 1], bf16, name="cpw2_sb")
                nc.gpsimd.dma_start(out=cpw2_sb, in_=cpw2_d[:, :])
                cpb1_sb = const.tile([16, 1], f32, name="cpb1_sb")
                nc.gpsimd.dma_start(out=cpb1_sb, in_=cpb1_d[:, :])
                cpb2_sb = const.tile([1, 1], f32, name="cpb2_sb")
                nc.gpsimd.dma_start(out=cpb2_sb, in_=cpb2_d[:, :])
                cph_sb = zscr.tile([16, BL], fp8, name="cph_sb", tag="zz")
                for ch in range(2):
                    cps = mm.tile([16, 512], f32, name="cps", tag="mm")
                    for ki in range(KB):
                        nc.tensor.matmul(
                            cps, lhsT=cpw1_sb[:, ki, :],
                            rhs=xT_sb[:, ki, ch * 512:(ch + 1) * 512],
                            start=(ki == 0), stop=False)
                    for kp in range(KQ):
                        nc.tensor.matmul(
                            cps, lhsT=cpw1q_sb[:, kp],
                            rhs=xq_sb[:, kp, :, ch * 512:(ch + 1) * 512],
                            start=False, stop=(kp == KQ - 1),
                            perf_mode=DR)
                    S.activation(cph_sb[:, ch * 512:(ch + 1) * 512], cps,
                                 AF.Relu, bias=cpb1_sb)
                sparts = []
                for ch in range(2):
                    c2p = mm.tile([1, 512], f32, name="c2p", tag="mm")
                    nc.tensor.matmul(c2p, lhsT=cpw2_sb,
                                     rhs=cph_sb[:16, ch * 512:(ch + 1) * 512],
                                     start=True, stop=True)
                    cpw = zscr.tile([1, 512], bf16, name="cpw", tag="zz")
                    spart = scal.tile([1, 1], f32, name=f"spart{ch}",
                                      tag=f"spart{ch}")
                    S.activation(cpw, c2p, AF.Sigmoid, bias=cpb2_sb,
                                 accum_out=spart)
                    sparts.append(spart)
                s_loc = scal.tile([1, 1], f32, name="s_loc", tag="s_loc")
                V.tensor_add(s_loc, sparts[0], sparts[1])
                cin = dram.tile([1, 1], f32, name="cin")
                cout = dram.tile([1, 1], f32, name="cout")
                nc.sync.dma_start(out=cin, in_=s_loc)
                nc.gpsimd.collective_compute(
                    "AllReduce", ALU.add,
                    replica_groups=[list(range(N_CORES))],
                    ins=[cin.opt()], outs=[cout.opt()])
                s_b = sc("s_b", (P, 1))
                nc.gpsimd.dma_start(out=s_b, in_=cout.to_broadcast([P, 1]))
                c_b = sc("c_b", (P, 1))
                V.tensor_scalar(out=c_b, in0=s_b,
                                scalar1=(MAX_C - MIN_C) / B_FULL,
                                scalar2=MIN_C, op0=ALU.mult, op1=ALU.add)
                V.tensor_scalar_min(out=c_b, in0=c_b, scalar1=MAX_C)
                V.tensor_scalar_max(out=c_b, in0=c_b, scalar1=MIN_C)
                negc_b = sc("negc_b", (P, 1))
                V.tensor_scalar_mul(out=negc_b, in0=c_b, scalar1=-1.0)
                twoc_b = sc("twoc_b", (P, 1))
                V.tensor_scalar_mul(out=twoc_b, in0=c_b, scalar1=2.0)
                neg2c_b = sc("neg2c_b", (P, 1))
                V.tensor_scalar_mul(out=neg2c_b, in0=c_b, scalar1=-2.0)
                c2_b = sc("c2_b", (P, 1))
                V.tensor_mul(c2_b, c_b, c_b)

        # still inside ph1 scope vars; reopened below for MM2
            # ---------- per-row scalar chain (batch-major [128, 4]) -------
            y2a = sacc.tile([P, BL], f32, name="y2a", tag="sacc")
            xya = sacc.tile([P, BL], f32, name="xya", tag="sacc")
            uT_sb = big.tile([P, KH, BL], bf16, name="uT_sb", tag="big")

            chain_stage = {}

            def scalar_chain(ch):
                hsl = slice(ch * 512, (ch + 1) * 512)

                def sch(name):
                    return sc(f"{name}_{ch}", (P, 4))

                # stats [1,512] rows -> batch-major [128,4] via 12 K=1
                # matmuls (zero DMAs: shared-semaphore DMA ordering would
                # serialize a gather behind the paced weight streams)
                stats_sb = chain_stage[ch]
                xyz_ps = bcp.tile([P, 12], f32, name=f"xyz_ps{ch}", tag="bc")
                for si, r in enumerate((0, 32, 64)):
                    for j in range(4):
                        nc.tensor.matmul(
                            xyz_ps[:, si * 4 + j:si * 4 + j + 1],
                            lhsT=stats_sb[r:r + 1, j * 128:(j + 1) * 128],
                            rhs=ones[r:r + 1, :], start=True, stop=True,
                            tile_position=(r, 0), skip_group_check=True)
                xyz_sb = sc(f"xyz_{ch}", (P, 12))
                S.copy(xyz_sb, xyz_ps)
                x2 = xyz_sb[:, 0:4]
                y2 = xyz_sb[:, 4:8]
                xy = xyz_sb[:, 8:12]
                w = sch("w")
                V.scalar_tensor_tensor(out=w, in0=xy, scalar=-2.0, in1=y2,
                                       op0=ALU.mult, op1=ALU.add)
                A1 = sch("A1")
                V.tensor_scalar(out=A1, in0=w, scalar1=c_b, scalar2=1.0,
                                op0=ALU.mult, op1=ALU.add)
                A2 = sch("A2")
                V.tensor_scalar(out=A2, in0=x2, scalar1=negc_b, scalar2=1.0,
                                op0=ALU.mult, op1=ALU.add)
                p1 = sch("p1")
                V.tensor_mul(p1, x2, y2)
                den = sch("den")
                V.tensor_scalar(out=den, in0=p1, scalar1=c2_b, scalar2=1.0,
                                op0=ALU.mult, op1=ALU.add)
                V.scalar_tensor_tensor(out=den, in0=xy, scalar=neg2c_b, in1=den,
                                       op0=ALU.mult, op1=ALU.add)
                V.tensor_scalar_add(out=den, in0=den, scalar1=EPS)
                D = sch("D")
                V.reciprocal(D, den)
                t1 = sch("t1")
                V.tensor_mul(t1, A1, A1)
                V.tensor_mul(t1, t1, x2)
                t2 = sch("t2")
                V.tensor_mul(t2, A1, A2)
                V.tensor_mul(t2, t2, xy)
                t3 = sch("t3")
                V.tensor_mul(t3, A2, A2)
                V.tensor_mul(t3, t3, y2)
                na2 = sch("na2")
                V.scalar_tensor_tensor(out=na2, in0=t2, scalar=-2.0, in1=t1,
                                       op0=ALU.mult, op1=ALU.add)
                V.tensor_add(na2, na2, t3)
                dsq = sch("dsq")
                V.tensor_mul(dsq, D, D)
                V.tensor_mul(na2, na2, dsq)
                # tm = tanh(T*artanh(sqrt(s)))/sqrt(s) with s = c*na2 is
                # analytic in s; deg-12 poly on s in [0.5, 0.996] (max rel
                # err 5.7e-4 over the c-envelope, 4.5e-5 in the actual
                # band) -- keeps the whole chain on DVE (no ACT table
                # thrash against MM2's sigmoids).
                s_t = sch("s_t")
                V.tensor_scalar(out=s_t, in0=na2, scalar1=c_b, scalar2=None,
                                op0=ALU.mult)
                V.tensor_scalar_min(out=s_t, in0=s_t, scalar1=0.996)
                V.tensor_scalar_max(out=s_t, in0=s_t, scalar1=0.5)
                V.tensor_scalar_add(out=s_t, in0=s_t, scalar1=-0.75)
                TM_POLY = (2227824.6408410813, 448871.7528227819,
                           -312401.2221121575, -56799.3889483669,
                           17050.8363088851, 2766.4915063889557,
                           -404.5213056958804, -51.417097735340924,
                           6.522608450512562, 1.2576027346248937,
                           0.3868927385367392, 0.32011781072746887,
                           0.839226248286217)
                tm = sch("tm")
                tmw = sch("tmw")
                V.tensor_scalar(out=tm, in0=s_t, scalar1=TM_POLY[0],
                                scalar2=TM_POLY[1], op0=ALU.mult, op1=ALU.add)
                for cf_ in TM_POLY[2:]:
                    V.tensor_mul(tmw, tm, s_t)
                    V.tensor_scalar_add(out=tm, in0=tmw, scalar1=cf_)
                s1_ = sch("s1_")
                V.tensor_mul(s1_, A1, x2)
                s2_ = sch("s2_")
                V.tensor_mul(s2_, A2, xy)
                ha = sch("ha")
                V.tensor_sub(ha, s2_, s1_)
                V.tensor_mul(ha, ha, D)
                hm = sch("hm")
                V.tensor_mul(hm, tm, ha)
                tsq = sch("tsq")
                V.tensor_mul(tsq, tm, tm)
                m2 = sch("m2")
                V.tensor_mul(m2, tsq, na2)
                w2s = sch("w2s")
                V.scalar_tensor_tensor(out=w2s, in0=hm, scalar=2.0, in1=m2,
                                       op0=ALU.mult, op1=ALU.add)
                B1 = sch("B1")
                V.tensor_scalar(out=B1, in0=w2s, scalar1=c_b, scalar2=1.0,
                                op0=ALU.mult, op1=ALU.add)
                p2 = sch("p2")
                V.tensor_mul(p2, x2, m2)
                den2 = sch("den2")
                V.tensor_scalar(out=den2, in0=p2, scalar1=c2_b, scalar2=1.0,
                                op0=ALU.mult, op1=ALU.add)
                V.scalar_tensor_tensor(out=den2, in0=hm, scalar=twoc_b, in1=den2,
                                       op0=ALU.mult, op1=ALU.add)
                V.tensor_scalar_add(out=den2, in0=den2, scalar1=EPS)
                D2 = sch("D2")
                V.reciprocal(D2, den2)
                g = sch("g")
                V.tensor_mul(g, A2, tm)
                V.tensor_mul(g, g, D)
                w3 = sch("w3")
                V.tensor_mul(w3, g, A1)
                V.tensor_sub(w3, B1, w3)
                alpha_bm = sc(f"alpha_bm_{ch}", (P, 4), bf16)
                V.tensor_mul(alpha_bm, w3, D2)
                w4 = sch("w4")
                V.tensor_mul(w4, g, A2)
                beta_bm = sc(f"beta_bm_{ch}", (P, 4), bf16)
                V.tensor_mul(beta_bm, w4, D2)
                # broadcast alpha/beta to [128, 512] feature-major with PE
                # only (no DMAs): transpose each batch-major column into a
                # diagonal block (psum row 32j, cols j*128..) of a zeroed
                # tile, then one ones-matmul sums rows -> full broadcast.
                tr_ps = bcp.tile([P, 2, 512], bf16, name=f"tr_ps{ch}",
                                 tag="tr")
                for half, src in ((0, alpha_bm), (1, beta_bm)):
                    for j in range(4):
                        nc.tensor.transpose(
                            tr_ps[32 * j:32 * j + 1, half,
                                  j * 128:(j + 1) * 128],
                            src[:, j:j + 1], ident[:, :],
                            tile_position=(0, 32 * j))
                for half, dst in ((0, alpha_b), (1, beta_b)):
                    tr_sb = scal.tile([P, 512], bf16,
                                      name=f"tr_sb{ch}_{half}", tag="tr_sb")
                    V.memset(tr_sb, 0.0)
                    for j in range(4):
                        S.copy(tr_sb[32 * j:32 * j + 1,
                                     j * 128:(j + 1) * 128],
                               tr_ps[32 * j:32 * j + 1, half,
                                     j * 128:(j + 1) * 128])
                    bc_ps = bcp.tile([P, 512], f32, name=f"bc_ps{ch}_{half}",
                                     tag="bc")
                    nc.tensor.matmul(bc_ps, lhsT=ones_row[:, :],
                                     rhs=tr_sb,
                                     start=True, stop=True,
                                     skip_group_check=True)
                    S.copy(dst[:, hsl], bc_ps)

            def mm2_mh(ch, mh):
                csl = slice(ch * 512, (ch + 1) * 512)
                ps = mm.tile([P, 512], f32, name="ps2", tag="mm")
                w2row = wp.tile([P, KP, 2, P], fp8, name="w2row", tag="w")
                nc.sync.dma_start(out=w2row, in_=w2_d[mh])
                for kp in range(KP):
                    nc.tensor.matmul(ps, lhsT=w2row[:, kp],
                                     rhs=hq_sb[:, kp, :, csl],
                                     start=(kp == 0), stop=(kp == KP - 1),
                                     perf_mode=DR)
                if with_b2:
                    S.activation(uT_sb[:, mh, csl], ps, AF.Sigmoid,
                                 bias=b2_sb[:, mh:mh + 1],
                                 scale=1.0 / W2_SCALE)
                else:
                    S.activation(uT_sb[:, mh, csl], ps, AF.Sigmoid,
                                 scale=1.0 / W2_SCALE)
                uu = scr.tile([P, 512], bf16, name="uu", tag="hh")
                S.activation(uu, uT_sb[:, mh, csl], AF.Square)
                hu = scr.tile([P, 512], bf16, name="hu", tag="hh")
                V.tensor_mul(hu, hT_sb[:, mh, csl], uT_sb[:, mh, csl])
                if mh == 0:
                    V.tensor_copy(y2a[:, csl], uu)
                    V.tensor_copy(xya[:, csl], hu)
                else:
                    V.tensor_add(y2a[:, csl], y2a[:, csl], uu)
                    V.tensor_add(xya[:, csl], xya[:, csl], hu)

            def mm2_stats(ch):
                csl = slice(ch * 512, (ch + 1) * 512)
                nc.tensor.matmul(stat_ps[ch][32:33, :], lhsT=ones,
                                 rhs=y2a[:, csl], start=True, stop=True,
                                 skip_group_check=True)
                nc.tensor.matmul(stat_ps[ch][64:65, :], lhsT=ones,
                                 rhs=xya[:, csl], start=True, stop=True,
                                 skip_group_check=True)
                stats_sb = scal.tile([P, 512], f32, name=f"stats_sb{ch}",
                                     tag="stats_sb")
                for r in (0, 32, 64):
                    S.copy(stats_sb[r:r + 1, :], stat_ps[ch][r:r + 1, :])
                chain_stage[ch] = stats_sb

            def zcomb(ch):
                csl = slice(ch * 512, (ch + 1) * 512)
                for kh in range(KH):
                    t1z = zscr.tile([P, 512], bf16, name="t1z", tag="zz")
                    V.tensor_mul(t1z, hT_sb[:, kh, csl], alpha_b[:, csl])
                    t2z = zscr.tile([P, 512], bf16, name="t2z", tag="zz")
                    V.tensor_mul(t2z, uT_sb[:, kh, csl], beta_b[:, csl])
                    V.tensor_add(uT_sb[:, kh, csl], t1z, t2z)

            # ---------- MM2 ch0 ----------
            with nc.named_scope("mm2a"):
                for mh in range(KH):
                    mm2_mh(0, mh)
                mm2_stats(0)
            # high_priority: schedule the chain compactly as soon as its
            # deps allow -- its DMA completions gate later same-semaphore
            # weight DMAs (shared-semaphore head-of-line blocking)
            with nc.named_scope("chain0"), tc.high_priority():
                scalar_chain(0)
            # ---------- MM2 ch1; scheduler interleaves zcomb0 ----------
            with nc.named_scope("mm2b"):
                for mh in range(18):
                    mm2_mh(1, mh)
                with nc.named_scope("zcomb0"):
                    zcomb(0)
                for mh in range(18, KH):
                    mm2_mh(1, mh)
                mm2_stats(1)
            with nc.named_scope("chain1"), tc.high_priority():
                scalar_chain(1)
        # ph1 psum pools (mm, stp, bcp) released here

        # ---------- MMo: out = z @ Wo; zcomb1 under mmo0 ------
        with ExitStack() as ph2:
            mmo = ph2.enter_context(tc.tile_pool(name="mmo", bufs=8,
                                                 space="PSUM"))

            def mmo_ch(ch):
                pso = [mmo.tile([P, 500], f32, name=f"pso{ch}_{i}",
                                tag="mmo") for i in range(8)]
                for kh in range(KH):
                    wot = wop.tile([P, OUT], bf16, name="wot", tag="wo")
                    nc.sync.dma_start(out=wot, in_=wo_d[kh])
                    for i in range(4):
                        b = ch * 4 + i
                        for och in range(2):
                            nc.tensor.matmul(
                                pso[i * 2 + och],
                                lhsT=uT_sb[:, kh, b * P:(b + 1) * P],
                                rhs=wot[:, och * 500:(och + 1) * 500],
                                start=(kh == 0), stop=(kh == KH - 1))
                for i in range(4):
                    b = ch * 4 + i
                    for och in range(2):
                        osl = slice(och * 500, (och + 1) * 500)
                        ob = outp.tile([P, 500], bf16, name="ob", tag="ob")
                        if och == 0:
                            S.copy(ob, pso[i * 2])
                        else:
                            V.tensor_copy(ob, pso[i * 2 + 1])
                        nc.gpsimd.dma_start(
                            out=out_d[b * P:(b + 1) * P, osl], in_=ob)

            with nc.named_scope("zcomb1"):
                zcomb(1)
            with nc.named_scope("mmo0"):
                mmo_ch(0)
            with nc.named_scope("mmo1"):
                mmo_ch(1)

    nc.compile()
    return nc


def _get_nc(with_b1, with_b2):
    for k, v in _nc_cache:
        if k == (with_b1, with_b2):
            return v
    nc = _build(with_b1, with_b2)
    _nc_cache.append(((with_b1, with_b2), nc))
    return nc


def kernel(x, W1, b1, W2, b2, Wo, bo, cp_w1, cp_b1, cp_w2, cp_b2,
           _trace=False, _tmpdir=None):
    x = np.asarray(x, dtype=np.float32)
    with_b1 = bool(np.any(b1))
    with_b2 = bool(np.any(b2))
    nc = _get_nc(with_b1, with_b2)

    W1f = np.asarray(W1, np.float32)
    # w1r[mh, p, ki, q] = W1[ki*128+p, mh*128+q]  (bf16 slices 0..KB-1)
    w1_t = np.ascontiguousarray(
        W1f[:KB * P].reshape(KB, P, KH, P).transpose(2, 1, 0, 3)).astype(BF)
    # w1q[mh, p, kp, j, q] = W1[(KB+2kp+j)*128+p, mh*128+q] * 16
    w1q_t = np.ascontiguousarray(
        (W1f[KB * P:] * np.float32(W1_SCALE))
        .reshape(KQ, 2, P, KH, P).transpose(3, 2, 0, 1, 4)).astype(E4)
    # w2r[mh, p, kp, j, q] = W2[(2*kp+j)*128+p, mh*128+q] * 256 in e4m3
    w2_t = np.ascontiguousarray(
        (np.asarray(W2, np.float32) * np.float32(W2_SCALE))
        .reshape(KP, 2, P, KH, P).transpose(3, 2, 0, 1, 4)
    ).astype(E4)
    wo_t = np.asarray(Wo, np.float32).reshape(KH, P, OUT).astype(BF)
    cpw1f = np.asarray(cp_w1, np.float32).T
    cpw1_t = np.ascontiguousarray(
        cpw1f[:KB * P].reshape(KB, P, 16)).astype(BF)
    cpw1q_t = np.ascontiguousarray(
        (cpw1f[KB * P:] * np.float32(W1_SCALE))
        .reshape(KQ, 2, P, 16).transpose(0, 2, 1, 3)).astype(E4)
    cpw2_t = np.asarray(cp_w2, np.float32).reshape(1, 16).T.astype(BF)
    cpw2_t = np.ascontiguousarray(cpw2_t)
    cpb1_t = np.asarray(cp_b1, np.float32).reshape(16, 1)
    cpb2_t = np.asarray(cp_b2, np.float32).reshape(1, 1)
    b1_t = np.ascontiguousarray(np.asarray(b1, np.float32).reshape(KH, P).T)
    b2_t = np.ascontiguousarray(np.asarray(b2, np.float32).reshape(KH, P).T)

    in_maps = []
    for c in range(N_CORES):
        shard = x[c * BL:(c + 1) * BL]
        shT = np.ascontiguousarray(shard.T)
        xT = shT[:KB * P].reshape(KB, P, BL).astype(BF)
        xq = np.ascontiguousarray(
            (shT[KB * P:] * np.float32(X_SCALE))
            .reshape(KQ, 2, P, BL).transpose(0, 2, 1, 3)).astype(E4)
        m = {"xT": xT, "xq": xq, "w1": w1_t, "w1q": w1q_t, "w2": w2_t,
             "wo": wo_t, "cpw1": cpw1_t, "cpw1q": cpw1q_t,
             "cpw2": cpw2_t, "cpb1": cpb1_t, "cpb2": cpb2_t}
        if with_b1:
            m["b1"] = b1_t
        if with_b2:
            m["b2"] = b2_t
        in_maps.append(m)

    kw = {}
    if _trace:
        kw = dict(trace=True, tmpdir=_tmpdir or tempfile.mkdtemp(prefix="cdk_"))
    res = run_bass_kernel_spmd(nc, in_maps, list(range(N_CORES)), **kw)

    out = np.concatenate([res.results[c]["out"] for c in range(N_CORES)], axis=0)
    bo = np.asarray(bo, np.float32)
    if np.any(bo):
        out = out + bo
    if _trace:
        kernel._last_result = res
    return out
